# revision 7
# baseline (speedup 1.0000x reference)
"""Bayesian multihead attention on 8 Trainium2 NeuronCores.

Sharding: core c handles batch b = c // 4 and head group g = c % 4
(heads 4g..4g+3, i.e. a 256-wide column slice of the 1024-dim embedding).
Each core runs the full pipeline for its (batch, head-group):
  - QKV projections from host-pre-transposed activations/weights
  - attention with scores kept transposed [j, i] (j = key pos, i = query pos)
  - softmax normalizer Z obtained for free via a ones-column appended to V
  - partial output projection against the sampled Bayesian weight slice
Host sums the 4 per-batch partial outputs / attention partials.
"""

import sys

import numpy as np

_TRN_REPO = "/opt/trn_rl_repo"

S = 2048
B = 2
E = 1024
H = 16
HD = 64
NCORES = 8
GROUPS = 4  # head groups (cores per batch)
HPC = H // GROUPS  # heads per core
CS = HPC * HD  # embedding column slice per core
IB = 1024  # query-index block size

_cache = {}


def _ensure_env():
    if _TRN_REPO not in sys.path:
        sys.path.insert(0, _TRN_REPO)
    _apply_drain_patch()


def _apply_drain_patch():
    """walrus CoreV3 codegen in this container accepts at most ONE sync-wait
    command per instruction, but TileContext._drain_and_barrier attaches one
    wait per logical proc to a single SP Drain. Split them across SP NOPs."""
    import concourse.mybir as mybir
    import concourse.tile as tile
    from concourse.vector_clock import ScopedClock

    if getattr(tile.TileContext, "_drain_waits_split", False):
        return

    def _patched(self, tick_clock, wait_clock):
        nc = self.nc
        probe = mybir.InstNoOp(name=nc.get_next_instruction_name(), ins=[], outs=[])
        probe.engine = mybir.EngineType.SP
        wait_clock.add_sem_waits(probe, ScopedClock({None: tick_clock.global_clock}))
        waits = list(probe.sync_info.on_wait) if probe.sync_info is not None else []
        for w in waits:
            inst = nc.sync.nop(nofuse=True)
            inst.ins.sync_info = mybir.SyncInfo(on_wait=[w], on_update=[])
        nc.sync.drain()
        nc.all_engine_barrier()
        assert self.sems is not None
        popped = nc._tile_sem_poison_stack.pop()
        assert popped is self._sem_poison
        nc.clear_and_free_semaphores(list(self.sems.allocated().values()))
        nc.all_engine_barrier()

    tile.TileContext._drain_and_barrier = _patched
    tile.TileContext._drain_waits_split = True


def _split_multi_waits(nc):
    """This walrus build accepts at most one sync-wait command per
    instruction. Move extra waits onto same-engine NOPs placed before the
    instruction (same semantics: engine queues execute in order)."""
    import concourse.mybir as mybir

    n = 0
    for fn in nc.m.functions:
        for bb in fn.blocks:
            out = []
            for inst in bb.instructions:
                si = inst.sync_info
                if si is not None and len(si.on_wait) > 1:
                    waits = list(si.on_wait)
                    for w in waits[:-1]:
                        nop = mybir.InstNoOp(name=f"WSPLIT-{n}", ins=[], outs=[])
                        n += 1
                        nop.engine = inst.engine
                        nop.sync_info = mybir.SyncInfo(on_wait=[w], on_update=[])
                        out.append(nop)
                    inst.sync_info = mybir.SyncInfo(
                        on_wait=[waits[-1]], on_update=list(si.on_update)
                    )
                out.append(inst)
            bb.instructions = out


def build_nc(S=S, E=E, CS=CS, HPC=HPC, IB=IB, for_hw=True):
    """Build the single-core bass program (same program on all cores).

    for_hw=True applies the walrus single-sync-wait workaround, which CoreSim
    cannot execute; pass False when the program is for simulation."""
    _ensure_env()
    from contextlib import ExitStack

    import concourse.bass as bass
    import concourse.mybir as mybir
    import concourse.tile as tile

    f32 = mybir.dt.float32
    f32r = mybir.dt.float32r
    bf16 = mybir.dt.bfloat16
    EXP = mybir.ActivationFunctionType.Exp
    IDN = mybir.ActivationFunctionType.Identity
    CPY = mybir.ActivationFunctionType.Copy
    MULT = mybir.AluOpType.mult
    ADD = mybir.AluOpType.add

    KC = E // 128  # contraction chunks for projections
    CT = CS // 128  # column tiles of the per-core slice
    NJT = S // 128  # key-position tiles
    NIB = S // IB  # query-index blocks
    SBK = min(512, S)  # matmul moving-block for fp32 (one PSUM bank)
    NBK = min(512, IB)

    nc = bass.Bass("TRN2", target_bir_lowering=False)

    xq = nc.dram_tensor("xq_t", [E, S], f32r, kind="ExternalInput")
    xk = nc.dram_tensor("xk_t", [E, S], f32r, kind="ExternalInput")
    xv = nc.dram_tensor("xv_t", [E, S], f32r, kind="ExternalInput")
    wq = nc.dram_tensor("wq_t", [E, CS], f32r, kind="ExternalInput")
    wk = nc.dram_tensor("wk_t", [E, CS], f32r, kind="ExternalInput")
    wv = nc.dram_tensor("wv_t", [E, CS], f32r, kind="ExternalInput")
    bq = nc.dram_tensor("bq", [CT, 128, 1], f32, kind="ExternalInput")
    bk = nc.dram_tensor("bk", [CT, 128, 1], f32, kind="ExternalInput")
    bv = nc.dram_tensor("bv", [1, CS], f32r, kind="ExternalInput")
    ones_in = nc.dram_tensor("ones_r", [1, 128], f32r, kind="ExternalInput")
    owm = nc.dram_tensor("owm_t", [CS, E], f32, kind="ExternalInput")
    owl = nc.dram_tensor("owl_t", [CS, E], f32, kind="ExternalInput")
    owe = nc.dram_tensor("owe_t", [CS, E], f32, kind="ExternalInput")

    out_part = nc.dram_tensor("out_part", [S, E], f32, kind="ExternalOutput")
    attn_part = nc.dram_tensor("attn_part", [S, S], bf16, kind="ExternalOutput")

    def r(ap):
        return ap

    with tile.TileContext(nc) as tc, ExitStack() as ctx:
        pers = ctx.enter_context(tc.tile_pool(name="pers", bufs=1))

        # ---- persistent tiles ----
        qT = [pers.tile([128, S], f32r, tag=f"qT{t}", name=f"qT{t}") for t in range(CT)]
        kT = [pers.tile([128, S], f32r, tag=f"kT{t}", name=f"kT{t}") for t in range(CT)]
        ctxT = [
            pers.tile([128, S], f32r, tag=f"ctxT{t}", name=f"ctxT{t}") for t in range(CT)
        ]
        owT = [
            pers.tile([128, E], f32r, tag=f"owT{t}", name=f"owT{t}") for t in range(CT)
        ]
        # V with a ones column per (jt, head): [128, NJT * HPC * 65]
        vones = pers.tile([128, NJT * HPC * 65], bf16, tag="vones", name="vones")
        ones_bf = pers.tile([1, 128], bf16, tag="ones_bf", name="ones_bf")
        ones_f = pers.tile([1, 128], f32r, tag="ones_f", name="ones_f")
        bq_sb = [
            pers.tile([128, 1], f32, tag=f"bq{t}", name=f"bq{t}") for t in range(CT)
        ]
        bk_sb = [
            pers.tile([128, 1], f32, tag=f"bk{t}", name=f"bk{t}") for t in range(CT)
        ]
        bv_bc = pers.tile([128, CS], f32, tag="bv_bc", name="bv_bc")

        nc.vector.memset(ones_bf[:], 1.0)
        nc.sync.dma_start(ones_f[:], ones_in[:])
        vheads = vones[:].rearrange("p (n c) -> p n c", c=65)
        nc.vector.memset(vheads[:, :, 64:65], 1.0)
        for t in range(CT):
            nc.sync.dma_start(bq_sb[t][:], bq[t])
            nc.sync.dma_start(bk_sb[t][:], bk[t])

        # ---- phase A: sample Bayesian output weight  ow = mean + eps*exp(lg)
        with (
            tc.tile_pool(name="owload", bufs=3) as ldp,
            tc.tile_pool(name="owtmp", bufs=2) as twp,
            tc.tile_pool(name="pbias", bufs=1, space="PSUM") as pbp,
        ):
            bv_row = ldp.tile([1, CS], f32r, tag="bvr", name="bv_row")
            nc.sync.dma_start(bv_row[:], bv[:])
            bv_ps = pbp.tile([128, CS], f32, tag="bvp", name="bv_ps")
            nc.tensor.matmul(bv_ps[:], r(ones_f[:]), r(bv_row[:]))
            nc.vector.tensor_copy(bv_bc[:], bv_ps[:])
            for t in range(CT):
                mt = ldp.tile([128, E], f32, tag="owm", name=f"owm{t}")
                lg = ldp.tile([128, E], f32, tag="owl", name=f"owl{t}")
                ep = ldp.tile([128, E], f32, tag="owe", name=f"owe{t}")
                nc.sync.dma_start(mt[:], owm[t * 128 : (t + 1) * 128, :])
                nc.sync.dma_start(lg[:], owl[t * 128 : (t + 1) * 128, :])
                nc.sync.dma_start(ep[:], owe[t * 128 : (t + 1) * 128, :])
                ex = twp.tile([128, E], f32, tag="ex", name=f"ex{t}")
                nc.scalar.activation(ex[:], lg[:], EXP)
                nc.vector.tensor_tensor(ex[:], ex[:], ep[:], MULT)
                nc.vector.tensor_tensor(owT[t][:], ex[:], mt[:], ADD)

        # ---- phase B: projections ----
        # q/k in transposed layout [c, s] (head dim on partitions)
        with (
            tc.tile_pool(name="wqk", bufs=1) as wqp,
            tc.tile_pool(name="xstage", bufs=3) as xsp,
            tc.tile_pool(name="pqk", bufs=1, space="PSUM") as pqk,
        ):
            for name, xdr, wdr, bias_sb, dstT in (
                ("q", xq, wq, bq_sb, qT),
                ("k", xk, wk, bk_sb, kT),
            ):
                w_sb = []
                for kc in range(KC):
                    wt = wqp.tile([128, CS], f32r, tag=f"w{kc}", name=f"w{name}{kc}")
                    nc.sync.dma_start(wt[:], wdr[kc * 128 : (kc + 1) * 128, :])
                    w_sb.append(wt)
                ps = [
                    pqk.tile([128, S], f32, tag=f"pqk{t}", name=f"p{name}{t}")
                    for t in range(CT)
                ]
                for kc in range(KC):
                    xc = xsp.tile([128, S], f32r, tag="xc", name=f"x{name}{kc}")
                    nc.sync.dma_start(xc[:], xdr[kc * 128 : (kc + 1) * 128, :])
                    for t in range(CT):
                        lhsT = w_sb[kc][:, t * 128 : (t + 1) * 128]
                        for sb in range(0, S, SBK):
                            nc.tensor.matmul(
                                ps[t][:, sb : sb + SBK],
                                r(lhsT),
                                r(xc[:, sb : sb + SBK]),
                                start=(kc == 0),
                                stop=(kc == KC - 1),
                            )
                for t in range(CT):
                    nc.scalar.activation(
                        dstT[t][:], ps[t][:], IDN, bias=bias_sb[t][:, 0:1]
                    )

        # v in natural layout [s, c] + bias broadcast + ones interleave
        with (
            tc.tile_pool(name="wv", bufs=1) as wvp,
            tc.tile_pool(name="xvstage", bufs=1) as xvp,
            tc.tile_pool(name="pv", bufs=4, space="PSUM") as pvp,
        ):
            wv_sb = []
            xv_sb = []
            for kc in range(KC):
                wt = wvp.tile([128, CS], f32r, tag=f"wv{kc}", name=f"wv{kc}")
                nc.sync.dma_start(wt[:], wv[kc * 128 : (kc + 1) * 128, :])
                wv_sb.append(wt)
                xc = xvp.tile([128, S], f32r, tag=f"xv{kc}", name=f"xv{kc}")
                nc.sync.dma_start(xc[:], xv[kc * 128 : (kc + 1) * 128, :])
                xv_sb.append(xc)
            for st in range(NJT):
                vps = pvp.tile([128, CS], f32, tag="vps", name=f"vps{st}")
                for kc in range(KC):
                    nc.tensor.matmul(
                        vps[:],
                        r(xv_sb[kc][:, st * 128 : (st + 1) * 128]),
                        r(wv_sb[kc][:]),
                        start=(kc == 0),
                        stop=(kc == KC - 1),
                    )
                # add bias, cast to bf16, write strided into vones (ones cols kept)
                dst = vheads[:, st * HPC : (st + 1) * HPC, 0:64]
                src = vps[:].rearrange("p (h d) -> p h d", d=64)
                bsrc = bv_bc[:].rearrange("p (h d) -> p h d", d=64)
                nc.vector.tensor_tensor(dst, src, bsrc, ADD)

        # ---- phase C: attention ----
        with (
            tc.tile_pool(name="expp", bufs=2 * NJT) as expp,
            tc.tile_pool(name="accp", bufs=NJT) as accp,
            tc.tile_pool(name="tmpp", bufs=3) as tmpp,
            tc.tile_pool(name="rzp", bufs=4) as rzp,
            tc.tile_pool(name="rzbp", bufs=3) as rzbp,
            tc.tile_pool(name="rzfp", bufs=2) as rzfp,
            tc.tile_pool(name="pmm", bufs=2, space="PSUM") as pmm,
            tc.tile_pool(name="pctx", bufs=2, space="PSUM") as pctx,
        ):
            for ib in range(NIB):
                acc = [
                    accp.tile([128, IB], bf16, tag="acc", name=f"acc{ib}_{jt}")
                    for jt in range(NJT)
                ]
                for hp in range(HPC // 2):
                    heads = (2 * hp, 2 * hp + 1)
                    exp_tiles = {}
                    for jt in range(NJT):
                        for h in heads:
                            tI, pO = h // 2, 64 * (h % 2)
                            s_ps = pmm.tile(
                                [128, IB], f32, tag="s_ps", name=f"s{ib}_{h}_{jt}"
                            )
                            lhsT = kT[tI][pO : pO + 64, jt * 128 : (jt + 1) * 128]
                            for nb in range(0, IB, NBK):
                                nc.tensor.matmul(
                                    s_ps[:, nb : nb + NBK],
                                    r(lhsT),
                                    r(qT[tI][pO : pO + 64, ib * IB + nb : ib * IB + nb + NBK]),
                                )
                            et = expp.tile(
                                [128, IB], bf16, tag="exp", name=f"e{ib}_{h}_{jt}"
                            )
                            nc.scalar.activation(et[:], s_ps[:], EXP, scale=0.125)
                            exp_tiles[h, jt] = et
                    rz_bf_pair = {}
                    for h in heads:
                        tI, pO = h // 2, 64 * (h % 2)
                        c_ps = pctx.tile([65, IB], f32, tag="ctx", name=f"c{ib}_{h}")
                        for jt in range(NJT):
                            lhsT = vheads[:, jt * HPC + h, 0:65]
                            for nb in range(0, IB, NBK):
                                nc.tensor.matmul(
                                    c_ps[:, nb : nb + NBK],
                                    lhsT,
                                    exp_tiles[h, jt][:, nb : nb + NBK],
                                    start=(jt == 0),
                                    stop=(jt == NJT - 1),
                                )
                        rzf = rzp.tile([1, IB], f32, tag="rz", name=f"rz{ib}_{h}")
                        nc.vector.reciprocal(rzf[:], c_ps[64:65, :])
                        rzb = rzp.tile([1, IB], bf16, tag="rzb", name=f"rzb{ib}_{h}")
                        nc.vector.tensor_copy(rzb[:], rzf[:])
                        rz_ps = pmm.tile(
                            [128, IB], f32, tag="s_ps", name=f"rzp{ib}_{h}"
                        )
                        for nb in range(0, IB, NBK):
                            nc.tensor.matmul(
                                rz_ps[:, nb : nb + NBK],
                                ones_bf[:],
                                rzb[:, nb : nb + NBK],
                            )
                        RZb = rzbp.tile([128, IB], bf16, tag="RZb", name=f"RZb{ib}_{h}")
                        nc.vector.tensor_copy(RZb[:], rz_ps[:])
                        RZf = rzfp.tile([128, IB], f32, tag="RZf", name=f"RZf{ib}_{h}")
                        nc.vector.tensor_copy(RZf[:], rz_ps[:])
                        rz_bf_pair[h] = RZb
                        # normalized context -> ctxT [c, s]
                        nc.vector.tensor_tensor(
                            ctxT[tI][pO : pO + 64, ib * IB : (ib + 1) * IB],
                            c_ps[0:64, :],
                            RZf[0:64, :],
                            MULT,
                        )
                    # attention average accumulation (bf16)
                    for jt in range(NJT):
                        for h in heads:
                            if hp == 0 and h == heads[0]:
                                nc.vector.tensor_tensor(
                                    acc[jt][:], exp_tiles[h, jt][:], rz_bf_pair[h][:], MULT
                                )
                            else:
                                tmp = tmpp.tile(
                                    [128, IB], bf16, tag="tmp", name=f"t{ib}_{h}_{jt}"
                                )
                                nc.vector.tensor_tensor(
                                    tmp[:], exp_tiles[h, jt][:], rz_bf_pair[h][:], MULT
                                )
                                nc.vector.tensor_tensor(
                                    acc[jt][:], acc[jt][:], tmp[:], ADD
                                )
                for jt in range(NJT):
                    nc.sync.dma_start(
                        attn_part[jt * 128 : (jt + 1) * 128, ib * IB : (ib + 1) * IB],
                        acc[jt][:],
                    )

        # ---- phase D: output projection (partial over this core's columns)
        with (
            tc.tile_pool(name="outp", bufs=3) as outp,
            tc.tile_pool(name="pop", bufs=4, space="PSUM") as popp,
        ):
            for st in range(NJT):
                o_sb = outp.tile([128, E], f32, tag="osb", name=f"osb{st}")
                for fb in range(0, E, 512):
                    op_ps = popp.tile([128, 512], f32, tag="op", name=f"op{st}_{fb}")
                    for cc in range(CT):
                        nc.tensor.matmul(
                            op_ps[:],
                            r(ctxT[cc][:, st * 128 : (st + 1) * 128]),
                            r(owT[cc][:, fb : fb + 512]),
                            start=(cc == 0),
                            stop=(cc == CT - 1),
                        )
                    nc.scalar.activation(o_sb[:, fb : fb + 512], op_ps[:], CPY)
                nc.sync.dma_start(out_part[st * 128 : (st + 1) * 128, :], o_sb[:])

    if for_hw:
        _split_multi_waits(nc)
    return nc


def make_in_maps(query, key, value, q_w, q_b, k_w, k_b, v_w, v_b,
                 o_w_mean, o_w_lgstd, eps, S=S, B=B, E=E, CS=CS, HPC=HPC):
    CT = CS // 128
    groups = E // CS
    xT = {}
    for b in range(B):
        xT[b] = (
            np.ascontiguousarray(query[:, b, :].T),
            np.ascontiguousarray(key[:, b, :].T),
            np.ascontiguousarray(value[:, b, :].T),
        )
    in_maps = []
    for c in range(B * groups):
        b, g = divmod(c, groups)
        cols = slice(g * CS, (g + 1) * CS)
        xq_t, xk_t, xv_t = xT[b]
        in_maps.append(
            {
                "xq_t": xq_t,
                "xk_t": xk_t,
                "xv_t": xv_t,
                "wq_t": np.ascontiguousarray(q_w[cols, :].T),
                "wk_t": np.ascontiguousarray(k_w[cols, :].T),
                "wv_t": np.ascontiguousarray(v_w[cols, :].T),
                "bq": np.ascontiguousarray(q_b[cols]).reshape(CT, 128, 1),
                "bk": np.ascontiguousarray(k_b[cols]).reshape(CT, 128, 1),
                "bv": np.ascontiguousarray(v_b[cols]).reshape(1, CS),
                "ones_r": np.ones((1, 128), np.float32),
                "owm_t": np.ascontiguousarray(o_w_mean[:, cols].T),
                "owl_t": np.ascontiguousarray(o_w_lgstd[:, cols].T),
                "owe_t": np.ascontiguousarray(eps[:, cols].T),
            }
        )
    return in_maps


def assemble(results, S=S, B=B, E=E):
    groups = NCORES // B
    out = np.zeros((S, B, E), np.float32)
    avg = np.zeros((B, S, S), np.float32)
    for c in range(NCORES):
        b = c // groups
        out[:, b, :] += results[c]["out_part"]
        avg[b] += results[c]["attn_part"].astype(np.float32).T
    avg /= H
    return out, avg


def kernel(query, key, value, q_w, q_b, k_w, k_b, v_w, v_b,
           o_w_mean, o_w_lgstd, eps):
    _ensure_env()
    from concourse.bass_utils import run_bass_kernel_spmd

    if "nc" not in _cache:
        _cache["nc"] = build_nc()
    nc = _cache["nc"]

    args = [query, key, value, q_w, q_b, k_w, k_b, v_w, v_b,
            o_w_mean, o_w_lgstd, eps]
    args = [np.asarray(a, np.float32) for a in args]
    in_maps = make_in_maps(*args)
    res = run_bass_kernel_spmd(nc, in_maps, core_ids=list(range(NCORES)))
    return assemble(res.results)


# revision 10
# speedup vs baseline: 1.2833x; 1.2833x over previous
"""Bayesian multihead attention on 8 Trainium2 NeuronCores.

Sharding: core c handles batch b = c // 4 and head group g = c % 4
(heads 4g..4g+3, i.e. a 256-wide column slice of the 1024-dim embedding).
Each core runs the full pipeline for its (batch, head-group):
  - QKV projections from host-pre-transposed activations/weights
  - attention with scores kept transposed [j, i] (j = key pos, i = query pos)
  - softmax normalizer Z obtained for free via a ones-column appended to V
  - partial output projection against the sampled Bayesian weight slice
Host sums the 4 per-batch partial outputs / attention partials.
"""

import sys

import numpy as np

_TRN_REPO = "/opt/trn_rl_repo"

S = 2048
B = 2
E = 1024
H = 16
HD = 64
NCORES = 8
GROUPS = 4  # head groups (cores per batch)
HPC = H // GROUPS  # heads per core
CS = HPC * HD  # embedding column slice per core
IB = 1024  # query-index block size

_cache = {}


def _ensure_env():
    if _TRN_REPO not in sys.path:
        sys.path.insert(0, _TRN_REPO)
    _apply_drain_patch()


def _apply_drain_patch():
    """walrus CoreV3 codegen in this container accepts at most ONE sync-wait
    command per instruction, but TileContext._drain_and_barrier attaches one
    wait per logical proc to a single SP Drain. Split them across SP NOPs."""
    import concourse.mybir as mybir
    import concourse.tile as tile
    from concourse.vector_clock import ScopedClock

    if getattr(tile.TileContext, "_drain_waits_split", False):
        return

    def _patched(self, tick_clock, wait_clock):
        nc = self.nc
        probe = mybir.InstNoOp(name=nc.get_next_instruction_name(), ins=[], outs=[])
        probe.engine = mybir.EngineType.SP
        wait_clock.add_sem_waits(probe, ScopedClock({None: tick_clock.global_clock}))
        waits = list(probe.sync_info.on_wait) if probe.sync_info is not None else []
        for w in waits:
            inst = nc.sync.nop(nofuse=True)
            inst.ins.sync_info = mybir.SyncInfo(on_wait=[w], on_update=[])
        nc.sync.drain()
        nc.all_engine_barrier()
        assert self.sems is not None
        popped = nc._tile_sem_poison_stack.pop()
        assert popped is self._sem_poison
        nc.clear_and_free_semaphores(list(self.sems.allocated().values()))
        nc.all_engine_barrier()

    tile.TileContext._drain_and_barrier = _patched
    tile.TileContext._drain_waits_split = True


def _split_multi_waits(nc):
    """This walrus build accepts at most one sync-wait command per
    instruction. Move extra waits onto same-engine NOPs placed before the
    instruction (same semantics: engine queues execute in order)."""
    import concourse.mybir as mybir

    n = 0
    for fn in nc.m.functions:
        for bb in fn.blocks:
            out = []
            for inst in bb.instructions:
                si = inst.sync_info
                if si is not None and len(si.on_wait) > 1:
                    waits = list(si.on_wait)
                    for w in waits[:-1]:
                        nop = mybir.InstNoOp(name=f"WSPLIT-{n}", ins=[], outs=[])
                        n += 1
                        nop.engine = inst.engine
                        nop.sync_info = mybir.SyncInfo(on_wait=[w], on_update=[])
                        out.append(nop)
                    inst.sync_info = mybir.SyncInfo(
                        on_wait=[waits[-1]], on_update=list(si.on_update)
                    )
                out.append(inst)
            bb.instructions = out


def build_nc(S=S, E=E, CS=CS, HPC=HPC, IB=IB, for_hw=True):
    """Build the single-core bass program (same program on all cores).

    for_hw=True applies the walrus single-sync-wait workaround, which CoreSim
    cannot execute; pass False when the program is for simulation."""
    _ensure_env()
    from contextlib import ExitStack

    import concourse.bass as bass
    import concourse.mybir as mybir
    import concourse.tile as tile

    f32 = mybir.dt.float32
    f32r = mybir.dt.float32r
    bf16 = mybir.dt.bfloat16
    EXP = mybir.ActivationFunctionType.Exp
    IDN = mybir.ActivationFunctionType.Identity
    CPY = mybir.ActivationFunctionType.Copy
    MULT = mybir.AluOpType.mult
    ADD = mybir.AluOpType.add

    KC = E // 128  # contraction chunks for projections
    CT = CS // 128  # column tiles of the per-core slice
    NJT = S // 128  # key-position tiles
    NIB = S // IB  # query-index blocks
    SBK = min(512, S)  # matmul moving-block for fp32 (one PSUM bank)
    NBK = min(512, IB)

    nc = bass.Bass("TRN2", target_bir_lowering=False)

    xq = nc.dram_tensor("xq_t", [E, S], bf16, kind="ExternalInput")
    xk = nc.dram_tensor("xk_t", [E, S], bf16, kind="ExternalInput")
    xv = nc.dram_tensor("xv_t", [E, S], bf16, kind="ExternalInput")
    wq = nc.dram_tensor("wq_t", [E, CS], bf16, kind="ExternalInput")
    wk = nc.dram_tensor("wk_t", [E, CS], bf16, kind="ExternalInput")
    wv = nc.dram_tensor("wv_t", [E, CS], bf16, kind="ExternalInput")
    bq = nc.dram_tensor("bq", [CT, 128, 1], f32, kind="ExternalInput")
    bk = nc.dram_tensor("bk", [CT, 128, 1], f32, kind="ExternalInput")
    bv = nc.dram_tensor("bv", [1, CS], f32r, kind="ExternalInput")
    ones_in = nc.dram_tensor("ones_r", [1, 128], f32r, kind="ExternalInput")
    owm = nc.dram_tensor("owm_t", [CS, E], f32, kind="ExternalInput")
    owl = nc.dram_tensor("owl_t", [CS, E], f32, kind="ExternalInput")
    owe = nc.dram_tensor("owe_t", [CS, E], f32, kind="ExternalInput")

    out_part = nc.dram_tensor("out_part", [S, E], f32, kind="ExternalOutput")
    attn_part = nc.dram_tensor("attn_part", [S, S], bf16, kind="ExternalOutput")

    def r(ap):
        return ap

    with tile.TileContext(nc) as tc, ExitStack() as ctx:
        pers = ctx.enter_context(tc.tile_pool(name="pers", bufs=1))

        # ---- persistent tiles ----
        qT = [pers.tile([128, S], bf16, tag=f"qT{t}", name=f"qT{t}") for t in range(CT)]
        kT = [pers.tile([128, S], bf16, tag=f"kT{t}", name=f"kT{t}") for t in range(CT)]
        ctxT = [
            pers.tile([128, S], f32r, tag=f"ctxT{t}", name=f"ctxT{t}") for t in range(CT)
        ]
        owT = [
            pers.tile([128, E], f32r, tag=f"owT{t}", name=f"owT{t}") for t in range(CT)
        ]
        # V with a ones column per (jt, head): [128, NJT * HPC * 65]
        vones = pers.tile([128, NJT * HPC * 65], bf16, tag="vones", name="vones")
        ones_bf = pers.tile([1, 128], bf16, tag="ones_bf", name="ones_bf")
        ones_f = pers.tile([1, 128], f32r, tag="ones_f", name="ones_f")
        bq_sb = [
            pers.tile([128, 1], f32, tag=f"bq{t}", name=f"bq{t}") for t in range(CT)
        ]
        bk_sb = [
            pers.tile([128, 1], f32, tag=f"bk{t}", name=f"bk{t}") for t in range(CT)
        ]
        bv_bc = pers.tile([128, CS], f32, tag="bv_bc", name="bv_bc")

        nc.vector.memset(ones_bf[:], 1.0)
        nc.sync.dma_start(ones_f[:], ones_in[:])
        vheads = vones[:].rearrange("p (n c) -> p n c", c=65)
        nc.vector.memset(vheads[:, :, 64:65], 1.0)
        for t in range(CT):
            nc.sync.dma_start(bq_sb[t][:], bq[t])
            nc.sync.dma_start(bk_sb[t][:], bk[t])

        # ---- phase A: sample Bayesian output weight  ow = mean + eps*exp(lg)
        with (
            tc.tile_pool(name="owload", bufs=3) as ldp,
            tc.tile_pool(name="owtmp", bufs=2) as twp,
            tc.tile_pool(name="pbias", bufs=1, space="PSUM") as pbp,
        ):
            bv_row = ldp.tile([1, CS], f32r, tag="bvr", name="bv_row")
            nc.sync.dma_start(bv_row[:], bv[:])
            bv_ps = pbp.tile([128, CS], f32, tag="bvp", name="bv_ps")
            nc.tensor.matmul(bv_ps[:], r(ones_f[:]), r(bv_row[:]))
            nc.vector.tensor_copy(bv_bc[:], bv_ps[:])
            for t in range(CT):
                mt = ldp.tile([128, E], f32, tag="owm", name=f"owm{t}")
                lg = ldp.tile([128, E], f32, tag="owl", name=f"owl{t}")
                ep = ldp.tile([128, E], f32, tag="owe", name=f"owe{t}")
                nc.sync.dma_start(mt[:], owm[t * 128 : (t + 1) * 128, :])
                nc.sync.dma_start(lg[:], owl[t * 128 : (t + 1) * 128, :])
                nc.sync.dma_start(ep[:], owe[t * 128 : (t + 1) * 128, :])
                ex = twp.tile([128, E], f32, tag="ex", name=f"ex{t}")
                nc.scalar.activation(ex[:], lg[:], EXP)
                nc.vector.tensor_tensor(ex[:], ex[:], ep[:], MULT)
                nc.vector.tensor_tensor(owT[t][:], ex[:], mt[:], ADD)

        # ---- phase B: projections ----
        # q/k in transposed layout [c, s] (head dim on partitions)
        with (
            tc.tile_pool(name="wqk", bufs=1) as wqp,
            tc.tile_pool(name="xstage", bufs=3) as xsp,
            tc.tile_pool(name="pqk", bufs=1, space="PSUM") as pqk,
        ):
            for name, xdr, wdr, bias_sb, dstT in (
                ("q", xq, wq, bq_sb, qT),
                ("k", xk, wk, bk_sb, kT),
            ):
                w_sb = []
                for kc in range(KC):
                    wt = wqp.tile([128, CS], bf16, tag=f"w{kc}", name=f"w{name}{kc}")
                    nc.sync.dma_start(wt[:], wdr[kc * 128 : (kc + 1) * 128, :])
                    w_sb.append(wt)
                ps = [
                    pqk.tile([128, S], f32, tag=f"pqk{t}", name=f"p{name}{t}")
                    for t in range(CT)
                ]
                for kc in range(KC):
                    xc = xsp.tile([128, S], bf16, tag="xc", name=f"x{name}{kc}")
                    nc.sync.dma_start(xc[:], xdr[kc * 128 : (kc + 1) * 128, :])
                    for t in range(CT):
                        lhsT = w_sb[kc][:, t * 128 : (t + 1) * 128]
                        for sb in range(0, S, SBK):
                            nc.tensor.matmul(
                                ps[t][:, sb : sb + SBK],
                                r(lhsT),
                                r(xc[:, sb : sb + SBK]),
                                start=(kc == 0),
                                stop=(kc == KC - 1),
                            )
                for t in range(CT):
                    nc.scalar.activation(
                        dstT[t][:], ps[t][:], IDN, bias=bias_sb[t][:, 0:1]
                    )

        # v in natural layout [s, c] + bias broadcast + ones interleave
        with (
            tc.tile_pool(name="wv", bufs=1) as wvp,
            tc.tile_pool(name="xvstage", bufs=1) as xvp,
            tc.tile_pool(name="pv", bufs=4, space="PSUM") as pvp,
        ):
            wv_sb = []
            xv_sb = []
            for kc in range(KC):
                wt = wvp.tile([128, CS], bf16, tag=f"wv{kc}", name=f"wv{kc}")
                nc.sync.dma_start(wt[:], wv[kc * 128 : (kc + 1) * 128, :])
                wv_sb.append(wt)
                xc = xvp.tile([128, S], bf16, tag=f"xv{kc}", name=f"xv{kc}")
                nc.sync.dma_start(xc[:], xv[kc * 128 : (kc + 1) * 128, :])
                xv_sb.append(xc)
            for st in range(NJT):
                vps = pvp.tile([128, CS], f32, tag="vps", name=f"vps{st}")
                for kc in range(KC):
                    nc.tensor.matmul(
                        vps[:],
                        r(xv_sb[kc][:, st * 128 : (st + 1) * 128]),
                        r(wv_sb[kc][:]),
                        start=(kc == 0),
                        stop=(kc == KC - 1),
                    )
                # add bias, cast to bf16, write strided into vones (ones cols kept)
                dst = vheads[:, st * HPC : (st + 1) * HPC, 0:64]
                src = vps[:].rearrange("p (h d) -> p h d", d=64)
                bsrc = bv_bc[:].rearrange("p (h d) -> p h d", d=64)
                nc.vector.tensor_tensor(dst, src, bsrc, ADD)

        # ---- phase C: attention ----
        with (
            tc.tile_pool(name="expp", bufs=2 * NJT + 8) as expp,
            tc.tile_pool(name="accp", bufs=NJT) as accp,
            tc.tile_pool(name="tmpp", bufs=4) as tmpp,
            tc.tile_pool(name="rzp", bufs=4) as rzp,
            tc.tile_pool(name="rzbp", bufs=3) as rzbp,
            tc.tile_pool(name="pmm", bufs=2, space="PSUM") as pmm,
            tc.tile_pool(name="pctx", bufs=2, space="PSUM") as pctx,
        ):
            for ib in range(NIB):
                acc = [
                    accp.tile([128, IB], bf16, tag="acc", name=f"acc{ib}_{jt}")
                    for jt in range(NJT)
                ]
                for hp in range(HPC // 2):
                    heads = (2 * hp, 2 * hp + 1)
                    exp_tiles = {}
                    for jt in range(NJT):
                        for h in heads:
                            tI, pO = h // 2, 64 * (h % 2)
                            s_ps = pmm.tile(
                                [128, IB], f32, tag="s_ps", name=f"s{ib}_{h}_{jt}"
                            )
                            lhsT = kT[tI][pO : pO + 64, jt * 128 : (jt + 1) * 128]
                            for nb in range(0, IB, NBK):
                                nc.tensor.matmul(
                                    s_ps[:, nb : nb + NBK],
                                    r(lhsT),
                                    r(qT[tI][pO : pO + 64, ib * IB + nb : ib * IB + nb + NBK]),
                                )
                            et = expp.tile(
                                [128, IB], bf16, tag="exp", name=f"e{ib}_{h}_{jt}"
                            )
                            nc.scalar.activation(et[:], s_ps[:], EXP, scale=0.125)
                            exp_tiles[h, jt] = et
                    for h in heads:
                        tI, pO = h // 2, 64 * (h % 2)
                        c_ps = pctx.tile([65, IB], f32, tag="ctx", name=f"c{ib}_{h}")
                        for jt in range(NJT):
                            lhsT = vheads[:, jt * HPC + h, 0:65]
                            for nb in range(0, IB, NBK):
                                nc.tensor.matmul(
                                    c_ps[:, nb : nb + NBK],
                                    lhsT,
                                    exp_tiles[h, jt][:, nb : nb + NBK],
                                    start=(jt == 0),
                                    stop=(jt == NJT - 1),
                                )
                        rzb = rzp.tile([1, IB], bf16, tag="rzb", name=f"rzb{ib}_{h}")
                        with nc.allow_low_precision(reason="1/Z in bf16 is enough"):
                            nc.vector.reciprocal(rzb[:], c_ps[64:65, :])
                        rz_ps = pmm.tile(
                            [128, IB], f32, tag="s_ps", name=f"rzp{ib}_{h}"
                        )
                        for nb in range(0, IB, NBK):
                            nc.tensor.matmul(
                                rz_ps[:, nb : nb + NBK],
                                ones_bf[:],
                                rzb[:, nb : nb + NBK],
                            )
                        RZb = rzbp.tile([128, IB], bf16, tag="RZb", name=f"RZb{ib}_{h}")
                        nc.vector.tensor_copy(RZb[:], rz_ps[:])
                        # normalized context -> ctxT [c, s]
                        nc.vector.tensor_tensor(
                            ctxT[tI][pO : pO + 64, ib * IB : (ib + 1) * IB],
                            c_ps[0:64, :],
                            RZb[0:64, :],
                            MULT,
                        )
                        # attention average accumulation (bf16) for this head
                        for jt in range(NJT):
                            if hp == 0 and h == heads[0]:
                                nc.vector.tensor_tensor(
                                    acc[jt][:], exp_tiles[h, jt][:], RZb[:], MULT
                                )
                            else:
                                tmp = tmpp.tile(
                                    [128, IB], bf16, tag="tmp", name=f"t{ib}_{h}_{jt}"
                                )
                                nc.vector.tensor_tensor(
                                    tmp[:], exp_tiles[h, jt][:], RZb[:], MULT
                                )
                                nc.vector.tensor_tensor(
                                    acc[jt][:], acc[jt][:], tmp[:], ADD
                                )
                for jt in range(NJT):
                    nc.sync.dma_start(
                        attn_part[jt * 128 : (jt + 1) * 128, ib * IB : (ib + 1) * IB],
                        acc[jt][:],
                    )

        # ---- phase D: output projection (partial over this core's columns)
        with (
            tc.tile_pool(name="outp", bufs=3) as outp,
            tc.tile_pool(name="pop", bufs=4, space="PSUM") as popp,
        ):
            for st in range(NJT):
                o_sb = outp.tile([128, E], f32, tag="osb", name=f"osb{st}")
                for fb in range(0, E, 512):
                    op_ps = popp.tile([128, 512], f32, tag="op", name=f"op{st}_{fb}")
                    for cc in range(CT):
                        nc.tensor.matmul(
                            op_ps[:],
                            r(ctxT[cc][:, st * 128 : (st + 1) * 128]),
                            r(owT[cc][:, fb : fb + 512]),
                            start=(cc == 0),
                            stop=(cc == CT - 1),
                        )
                    nc.scalar.activation(o_sb[:, fb : fb + 512], op_ps[:], CPY)
                nc.sync.dma_start(out_part[st * 128 : (st + 1) * 128, :], o_sb[:])

    if for_hw:
        _split_multi_waits(nc)
    return nc


def make_in_maps(query, key, value, q_w, q_b, k_w, k_b, v_w, v_b,
                 o_w_mean, o_w_lgstd, eps, S=S, B=B, E=E, CS=CS, HPC=HPC):
    CT = CS // 128
    groups = E // CS
    import ml_dtypes

    _bf = ml_dtypes.bfloat16
    xT = {}
    for b in range(B):
        xT[b] = (
            np.ascontiguousarray(query[:, b, :].T).astype(_bf),
            np.ascontiguousarray(key[:, b, :].T).astype(_bf),
            np.ascontiguousarray(value[:, b, :].T).astype(_bf),
        )
    in_maps = []
    for c in range(B * groups):
        b, g = divmod(c, groups)
        cols = slice(g * CS, (g + 1) * CS)
        xq_t, xk_t, xv_t = xT[b]
        in_maps.append(
            {
                "xq_t": xq_t,
                "xk_t": xk_t,
                "xv_t": xv_t,
                "wq_t": np.ascontiguousarray(q_w[cols, :].T).astype(_bf),
                "wk_t": np.ascontiguousarray(k_w[cols, :].T).astype(_bf),
                "wv_t": np.ascontiguousarray(v_w[cols, :].T).astype(_bf),
                "bq": np.ascontiguousarray(q_b[cols]).reshape(CT, 128, 1),
                "bk": np.ascontiguousarray(k_b[cols]).reshape(CT, 128, 1),
                "bv": np.ascontiguousarray(v_b[cols]).reshape(1, CS),
                "ones_r": np.ones((1, 128), np.float32),
                "owm_t": np.ascontiguousarray(o_w_mean[:, cols].T),
                "owl_t": np.ascontiguousarray(o_w_lgstd[:, cols].T),
                "owe_t": np.ascontiguousarray(eps[:, cols].T),
            }
        )
    return in_maps


def assemble(results, S=S, B=B, E=E):
    groups = NCORES // B
    out = np.zeros((S, B, E), np.float32)
    avg = np.zeros((B, S, S), np.float32)
    for c in range(NCORES):
        b = c // groups
        out[:, b, :] += results[c]["out_part"]
        avg[b] += results[c]["attn_part"].astype(np.float32).T
    avg /= H
    return out, avg


def kernel(query, key, value, q_w, q_b, k_w, k_b, v_w, v_b,
           o_w_mean, o_w_lgstd, eps):
    _ensure_env()
    from concourse.bass_utils import run_bass_kernel_spmd

    if "nc" not in _cache:
        _cache["nc"] = build_nc()
    nc = _cache["nc"]

    args = [query, key, value, q_w, q_b, k_w, k_b, v_w, v_b,
            o_w_mean, o_w_lgstd, eps]
    args = [np.asarray(a, np.float32) for a in args]
    in_maps = make_in_maps(*args)
    res = run_bass_kernel_spmd(nc, in_maps, core_ids=list(range(NCORES)))
    return assemble(res.results)


# revision 17
# speedup vs baseline: 1.2879x; 1.0035x over previous
"""Bayesian multihead attention on 8 Trainium2 NeuronCores.

Sharding: core c handles batch b = c // 4 and head group g = c % 4
(heads 4g..4g+3, i.e. a 256-wide column slice of the 1024-dim embedding).
Each core runs the full pipeline for its (batch, head-group):
  - QKV projections from host-pre-transposed activations/weights
  - attention with scores kept transposed [j, i] (j = key pos, i = query pos)
  - softmax normalizer Z obtained for free via a ones-column appended to V
  - partial output projection against the sampled Bayesian weight slice
Host sums the 4 per-batch partial outputs / attention partials.
"""

import sys

import numpy as np

_TRN_REPO = "/opt/trn_rl_repo"

S = 2048
B = 2
E = 1024
H = 16
HD = 64
NCORES = 8
GROUPS = 4  # head groups (cores per batch)
HPC = H // GROUPS  # heads per core
CS = HPC * HD  # embedding column slice per core
IB = 1024  # query-index block size

_cache = {}


def _ensure_env():
    if _TRN_REPO not in sys.path:
        sys.path.insert(0, _TRN_REPO)
    _apply_drain_patch()


def _apply_drain_patch():
    """walrus CoreV3 codegen in this container accepts at most ONE sync-wait
    command per instruction, but TileContext._drain_and_barrier attaches one
    wait per logical proc to a single SP Drain. Split them across SP NOPs."""
    import concourse.mybir as mybir
    import concourse.tile as tile
    from concourse.vector_clock import ScopedClock

    if getattr(tile.TileContext, "_drain_waits_split", False):
        return

    def _patched(self, tick_clock, wait_clock):
        nc = self.nc
        probe = mybir.InstNoOp(name=nc.get_next_instruction_name(), ins=[], outs=[])
        probe.engine = mybir.EngineType.SP
        wait_clock.add_sem_waits(probe, ScopedClock({None: tick_clock.global_clock}))
        waits = list(probe.sync_info.on_wait) if probe.sync_info is not None else []
        for w in waits:
            inst = nc.sync.nop(nofuse=True)
            inst.ins.sync_info = mybir.SyncInfo(on_wait=[w], on_update=[])
        nc.sync.drain()
        nc.all_engine_barrier()
        assert self.sems is not None
        popped = nc._tile_sem_poison_stack.pop()
        assert popped is self._sem_poison
        nc.clear_and_free_semaphores(list(self.sems.allocated().values()))
        nc.all_engine_barrier()

    tile.TileContext._drain_and_barrier = _patched
    tile.TileContext._drain_waits_split = True


def _split_multi_waits(nc):
    """This walrus build accepts at most one sync-wait command per
    instruction. Move extra waits onto same-engine NOPs placed before the
    instruction (same semantics: engine queues execute in order)."""
    import concourse.mybir as mybir

    n = 0
    for fn in nc.m.functions:
        for bb in fn.blocks:
            out = []
            for inst in bb.instructions:
                si = inst.sync_info
                if si is not None and len(si.on_wait) > 1:
                    waits = list(si.on_wait)
                    for w in waits[:-1]:
                        nop = mybir.InstNoOp(name=f"WSPLIT-{n}", ins=[], outs=[])
                        n += 1
                        nop.engine = inst.engine
                        nop.sync_info = mybir.SyncInfo(on_wait=[w], on_update=[])
                        out.append(nop)
                    inst.sync_info = mybir.SyncInfo(
                        on_wait=[waits[-1]], on_update=list(si.on_update)
                    )
                out.append(inst)
            bb.instructions = out


def build_nc(S=S, E=E, CS=CS, HPC=HPC, IB=IB, for_hw=True):
    """Build the single-core bass program (same program on all cores).

    for_hw=True applies the walrus single-sync-wait workaround, which CoreSim
    cannot execute; pass False when the program is for simulation."""
    _ensure_env()
    from contextlib import ExitStack

    import concourse.bass as bass
    import concourse.mybir as mybir
    import concourse.tile as tile

    f32 = mybir.dt.float32
    f32r = mybir.dt.float32r
    bf16 = mybir.dt.bfloat16
    EXP = mybir.ActivationFunctionType.Exp
    IDN = mybir.ActivationFunctionType.Identity
    CPY = mybir.ActivationFunctionType.Copy
    MULT = mybir.AluOpType.mult
    ADD = mybir.AluOpType.add

    KC = E // 128  # contraction chunks for projections
    CT = CS // 128  # column tiles of the per-core slice
    NJT = S // 128  # key-position tiles
    NIB = S // IB  # query-index blocks
    SBK = min(512, S)  # matmul moving-block for fp32 (one PSUM bank)
    NBK = min(512, IB)

    nc = bass.Bass("TRN2", target_bir_lowering=False)

    xq = nc.dram_tensor("xq_t", [E, S], bf16, kind="ExternalInput")
    xk = nc.dram_tensor("xk_t", [E, S], bf16, kind="ExternalInput")
    xv = nc.dram_tensor("xv_t", [E, S], bf16, kind="ExternalInput")
    wq = nc.dram_tensor("wq_t", [E, CS], bf16, kind="ExternalInput")
    wk = nc.dram_tensor("wk_t", [E, CS], bf16, kind="ExternalInput")
    wv = nc.dram_tensor("wv_t", [E, CS], bf16, kind="ExternalInput")
    bq = nc.dram_tensor("bq", [CT, 128, 1], f32, kind="ExternalInput")
    bk = nc.dram_tensor("bk", [CT, 128, 1], f32, kind="ExternalInput")
    bv = nc.dram_tensor("bv", [1, CS], f32r, kind="ExternalInput")
    ones_in = nc.dram_tensor("ones_r", [1, 128], f32r, kind="ExternalInput")
    owm = nc.dram_tensor("owm_t", [CS, E], f32, kind="ExternalInput")
    owl = nc.dram_tensor("owl_t", [CS, E], f32, kind="ExternalInput")
    owe = nc.dram_tensor("owe_t", [CS, E], f32, kind="ExternalInput")

    out_part = nc.dram_tensor("out_part", [S, E], f32, kind="ExternalOutput")
    attn_part = nc.dram_tensor("attn_part", [S, S], bf16, kind="ExternalOutput")

    def r(ap):
        return ap

    with tile.TileContext(nc) as tc, ExitStack() as ctx:
        pers = ctx.enter_context(tc.tile_pool(name="pers", bufs=1))

        # ---- persistent tiles ----
        qT = [pers.tile([128, S], bf16, tag=f"qT{t}", name=f"qT{t}") for t in range(CT)]
        kT = [pers.tile([128, S], bf16, tag=f"kT{t}", name=f"kT{t}") for t in range(CT)]
        ctxT = [
            pers.tile([128, S], f32r, tag=f"ctxT{t}", name=f"ctxT{t}") for t in range(CT)
        ]
        owT = [
            pers.tile([128, E], f32r, tag=f"owT{t}", name=f"owT{t}") for t in range(CT)
        ]
        # V with a ones column per (jt, head): [128, NJT * HPC * 65]
        vones = pers.tile([128, NJT * HPC * 65], bf16, tag="vones", name="vones")
        ones_bf = pers.tile([1, 128], bf16, tag="ones_bf", name="ones_bf")
        ones_f = pers.tile([1, 128], f32r, tag="ones_f", name="ones_f")
        bq_sb = [
            pers.tile([128, 1], f32, tag=f"bq{t}", name=f"bq{t}") for t in range(CT)
        ]
        bk_sb = [
            pers.tile([128, 1], f32, tag=f"bk{t}", name=f"bk{t}") for t in range(CT)
        ]
        bv_bc = pers.tile([128, CS], f32, tag="bv_bc", name="bv_bc")

        nc.vector.memset(ones_bf[:], 1.0)
        nc.sync.dma_start(ones_f[:], ones_in[:])
        vheads = vones[:].rearrange("p (n c) -> p n c", c=65)
        nc.vector.memset(vheads[:, :, 64:65], 1.0)
        for t in range(CT):
            nc.sync.dma_start(bq_sb[t][:], bq[t])
            nc.sync.dma_start(bk_sb[t][:], bk[t])

        # ---- phase A: sample Bayesian output weight  ow = mean + eps*exp(lg)
        with (
            tc.tile_pool(name="owload", bufs=3) as ldp,
            tc.tile_pool(name="owtmp", bufs=2) as twp,
            tc.tile_pool(name="pbias", bufs=1, space="PSUM") as pbp,
        ):
            bv_row = ldp.tile([1, CS], f32r, tag="bvr", name="bv_row")
            nc.sync.dma_start(bv_row[:], bv[:])
            bv_ps = pbp.tile([128, CS], f32, tag="bvp", name="bv_ps")
            nc.tensor.matmul(bv_ps[:], r(ones_f[:]), r(bv_row[:]))
            nc.vector.tensor_copy(bv_bc[:], bv_ps[:])
            for t in range(CT):
                mt = ldp.tile([128, E], f32, tag="owm", name=f"owm{t}")
                lg = ldp.tile([128, E], f32, tag="owl", name=f"owl{t}")
                ep = ldp.tile([128, E], f32, tag="owe", name=f"owe{t}")
                nc.sync.dma_start(mt[:], owm[t * 128 : (t + 1) * 128, :])
                nc.sync.dma_start(lg[:], owl[t * 128 : (t + 1) * 128, :])
                nc.sync.dma_start(ep[:], owe[t * 128 : (t + 1) * 128, :])
                ex = twp.tile([128, E], f32, tag="ex", name=f"ex{t}")
                nc.scalar.activation(ex[:], lg[:], EXP)
                nc.vector.tensor_tensor(ex[:], ex[:], ep[:], MULT)
                nc.vector.tensor_tensor(owT[t][:], ex[:], mt[:], ADD)

        # ---- phase B: projections ----
        # q/k in transposed layout [c, s] (head dim on partitions)
        with (
            tc.tile_pool(name="wqk", bufs=1) as wqp,
            tc.tile_pool(name="xstage", bufs=3) as xsp,
            tc.tile_pool(name="pqk", bufs=1, space="PSUM") as pqk,
        ):
            for name, xdr, wdr, bias_sb, dstT in (
                ("q", xq, wq, bq_sb, qT),
                ("k", xk, wk, bk_sb, kT),
            ):
                w_sb = []
                for kc in range(KC):
                    wt = wqp.tile([128, CS], bf16, tag=f"w{kc}", name=f"w{name}{kc}")
                    nc.sync.dma_start(wt[:], wdr[kc * 128 : (kc + 1) * 128, :])
                    w_sb.append(wt)
                ps = [
                    pqk.tile([128, S], f32, tag=f"pqk{t}", name=f"p{name}{t}")
                    for t in range(CT)
                ]
                for kc in range(KC):
                    xc = xsp.tile([128, S], bf16, tag="xc", name=f"x{name}{kc}")
                    nc.sync.dma_start(xc[:], xdr[kc * 128 : (kc + 1) * 128, :])
                    for t in range(CT):
                        lhsT = w_sb[kc][:, t * 128 : (t + 1) * 128]
                        for sb in range(0, S, SBK):
                            nc.tensor.matmul(
                                ps[t][:, sb : sb + SBK],
                                r(lhsT),
                                r(xc[:, sb : sb + SBK]),
                                start=(kc == 0),
                                stop=(kc == KC - 1),
                            )
                for t in range(CT):
                    nc.scalar.activation(
                        dstT[t][:], ps[t][:], IDN, bias=bias_sb[t][:, 0:1]
                    )

        # v in natural layout [s, c] + bias broadcast + ones interleave
        with (
            tc.tile_pool(name="wv", bufs=1) as wvp,
            tc.tile_pool(name="xvstage", bufs=1) as xvp,
            tc.tile_pool(name="pv", bufs=4, space="PSUM") as pvp,
        ):
            wv_sb = []
            xv_sb = []
            for kc in range(KC):
                wt = wvp.tile([128, CS], bf16, tag=f"wv{kc}", name=f"wv{kc}")
                nc.sync.dma_start(wt[:], wv[kc * 128 : (kc + 1) * 128, :])
                wv_sb.append(wt)
                xc = xvp.tile([128, S], bf16, tag=f"xv{kc}", name=f"xv{kc}")
                nc.sync.dma_start(xc[:], xv[kc * 128 : (kc + 1) * 128, :])
                xv_sb.append(xc)
            for st in range(NJT):
                vps = pvp.tile([128, CS], f32, tag="vps", name=f"vps{st}")
                for kc in range(KC):
                    nc.tensor.matmul(
                        vps[:],
                        r(xv_sb[kc][:, st * 128 : (st + 1) * 128]),
                        r(wv_sb[kc][:]),
                        start=(kc == 0),
                        stop=(kc == KC - 1),
                    )
                # add bias, cast to bf16, write strided into vones (ones cols kept)
                dst = vheads[:, st * HPC : (st + 1) * HPC, 0:64]
                src = vps[:].rearrange("p (h d) -> p h d", d=64)
                bsrc = bv_bc[:].rearrange("p (h d) -> p h d", d=64)
                nc.vector.tensor_tensor(dst, src, bsrc, ADD)

        # ---- phase C: attention ----
        with (
            tc.tile_pool(name="expp", bufs=2 * NJT + 7) as expp,
            tc.tile_pool(name="accp", bufs=NJT) as accp,
            tc.tile_pool(name="tmpp", bufs=4) as tmpp,
            tc.tile_pool(name="rzp", bufs=4) as rzp,
            tc.tile_pool(name="rzbp", bufs=3) as rzbp,
            tc.tile_pool(name="outp", bufs=2) as outp,
            tc.tile_pool(name="pmm", bufs=2, space="PSUM") as pmm,
            tc.tile_pool(name="pctx", bufs=2, space="PSUM") as pctx,
        ):
            def emit_outproj(ib):
                for st in range(ib * IB // 128, (ib + 1) * IB // 128):
                    o_sb = outp.tile([128, E], f32, tag="osb", name=f"osb{st}")
                    OPW = min(512, E)
                    for fb in range(0, E, OPW):
                        op_ps = pmm.tile([128, OPW], f32, tag="s_ps", name=f"op{st}_{fb}")
                        for cc in range(CT):
                            nc.tensor.matmul(
                                op_ps[:],
                                r(ctxT[cc][:, st * 128 : (st + 1) * 128]),
                                r(owT[cc][:, fb : fb + OPW]),
                                start=(cc == 0),
                                stop=(cc == CT - 1),
                            )
                        nc.scalar.activation(o_sb[:, fb : fb + OPW], op_ps[:], CPY)
                    nc.sync.dma_start(out_part[st * 128 : (st + 1) * 128, :], o_sb[:])

            pending_op = None
            for ib in range(NIB):
                acc = [
                    accp.tile([128, IB], bf16, tag="acc", name=f"acc{ib}_{jt}")
                    for jt in range(NJT)
                ]
                for hp in range(HPC // 2):
                    if hp == 1 and pending_op is not None:
                        emit_outproj(pending_op)
                        pending_op = None
                    heads = (2 * hp, 2 * hp + 1)
                    exp_tiles = {}
                    for jt in range(NJT):
                        for h in heads:
                            tI, pO = h // 2, 64 * (h % 2)
                            s_ps = pmm.tile(
                                [128, IB], f32, tag="s_ps", name=f"s{ib}_{h}_{jt}"
                            )
                            lhsT = kT[tI][pO : pO + 64, jt * 128 : (jt + 1) * 128]
                            for nb in range(0, IB, NBK):
                                nc.tensor.matmul(
                                    s_ps[:, nb : nb + NBK],
                                    r(lhsT),
                                    r(qT[tI][pO : pO + 64, ib * IB + nb : ib * IB + nb + NBK]),
                                )
                            et = expp.tile(
                                [128, IB], bf16, tag="exp", name=f"e{ib}_{h}_{jt}"
                            )
                            nc.scalar.activation(et[:], s_ps[:], EXP, scale=0.125)
                            exp_tiles[h, jt] = et
                    for h in heads:
                        tI, pO = h // 2, 64 * (h % 2)
                        c_ps = pctx.tile([65, IB], f32, tag="ctx", name=f"c{ib}_{h}")
                        for jt in range(NJT):
                            lhsT = vheads[:, jt * HPC + h, 0:65]
                            for nb in range(0, IB, NBK):
                                nc.tensor.matmul(
                                    c_ps[:, nb : nb + NBK],
                                    lhsT,
                                    exp_tiles[h, jt][:, nb : nb + NBK],
                                    start=(jt == 0),
                                    stop=(jt == NJT - 1),
                                )
                        # 1/Z: DMA-reshape the Z row across partitions so the
                        # DVE reciprocal runs 128 lanes wide, then reshape back
                        zrow = rzp.tile([1, IB], f32, tag="zrow", name=f"zrow{ib}_{h}")
                        nc.scalar.activation(zrow[:], c_ps[64:65, :], CPY)
                        zm = rzp.tile([128, IB // 128], f32, tag="zm", name=f"zm{ib}_{h}")
                        nc.sync.dma_start(zm[:], zrow[:])
                        zr = rzp.tile([128, IB // 128], bf16, tag="zr", name=f"zr{ib}_{h}")
                        with nc.allow_low_precision(reason="1/Z in bf16 is enough"):
                            nc.vector.reciprocal(zr[:], zm[:])
                        rzb = rzp.tile([1, IB], bf16, tag="rzb", name=f"rzb{ib}_{h}")
                        nc.sync.dma_start(rzb[:], zr[:])
                        rz_ps = pmm.tile(
                            [128, IB], f32, tag="s_ps", name=f"rzp{ib}_{h}"
                        )
                        for nb in range(0, IB, NBK):
                            nc.tensor.matmul(
                                rz_ps[:, nb : nb + NBK],
                                ones_bf[:],
                                rzb[:, nb : nb + NBK],
                            )
                        RZb = rzbp.tile([128, IB], bf16, tag="RZb", name=f"RZb{ib}_{h}")
                        nc.vector.tensor_copy(RZb[:], rz_ps[:])
                        # normalized context -> ctxT [c, s]
                        nc.vector.tensor_tensor(
                            ctxT[tI][pO : pO + 64, ib * IB : (ib + 1) * IB],
                            c_ps[0:64, :],
                            RZb[0:64, :],
                            MULT,
                        )
                        # attention average accumulation (bf16) for this head
                        for jt in range(NJT):
                            if hp == 0 and h == heads[0]:
                                nc.vector.tensor_tensor(
                                    acc[jt][:], exp_tiles[h, jt][:], RZb[:], MULT
                                )
                            else:
                                tmp = tmpp.tile(
                                    [128, IB], bf16, tag="tmp", name=f"t{ib}_{h}_{jt}"
                                )
                                nc.vector.tensor_tensor(
                                    tmp[:], exp_tiles[h, jt][:], RZb[:], MULT
                                )
                                nc.vector.tensor_tensor(
                                    acc[jt][:], acc[jt][:], tmp[:], ADD
                                )
                for jt in range(NJT):
                    nc.sync.dma_start(
                        attn_part[jt * 128 : (jt + 1) * 128, ib * IB : (ib + 1) * IB],
                        acc[jt][:],
                    )
                if pending_op is not None:
                    emit_outproj(pending_op)
                pending_op = ib
            emit_outproj(pending_op)


    if for_hw:
        _split_multi_waits(nc)
    return nc


def make_in_maps(query, key, value, q_w, q_b, k_w, k_b, v_w, v_b,
                 o_w_mean, o_w_lgstd, eps, S=S, B=B, E=E, CS=CS, HPC=HPC):
    CT = CS // 128
    groups = E // CS
    import ml_dtypes

    _bf = ml_dtypes.bfloat16
    xT = {}
    for b in range(B):
        xT[b] = (
            np.ascontiguousarray(query[:, b, :].T).astype(_bf),
            np.ascontiguousarray(key[:, b, :].T).astype(_bf),
            np.ascontiguousarray(value[:, b, :].T).astype(_bf),
        )
    in_maps = []
    for c in range(B * groups):
        b, g = divmod(c, groups)
        cols = slice(g * CS, (g + 1) * CS)
        xq_t, xk_t, xv_t = xT[b]
        in_maps.append(
            {
                "xq_t": xq_t,
                "xk_t": xk_t,
                "xv_t": xv_t,
                "wq_t": np.ascontiguousarray(q_w[cols, :].T).astype(_bf),
                "wk_t": np.ascontiguousarray(k_w[cols, :].T).astype(_bf),
                "wv_t": np.ascontiguousarray(v_w[cols, :].T).astype(_bf),
                "bq": np.ascontiguousarray(q_b[cols]).reshape(CT, 128, 1),
                "bk": np.ascontiguousarray(k_b[cols]).reshape(CT, 128, 1),
                "bv": np.ascontiguousarray(v_b[cols]).reshape(1, CS),
                "ones_r": np.ones((1, 128), np.float32),
                "owm_t": np.ascontiguousarray(o_w_mean[:, cols].T),
                "owl_t": np.ascontiguousarray(o_w_lgstd[:, cols].T),
                "owe_t": np.ascontiguousarray(eps[:, cols].T),
            }
        )
    return in_maps


def assemble(results, S=S, B=B, E=E):
    groups = NCORES // B
    out = np.zeros((S, B, E), np.float32)
    avg = np.zeros((B, S, S), np.float32)
    for c in range(NCORES):
        b = c // groups
        out[:, b, :] += results[c]["out_part"]
        avg[b] += results[c]["attn_part"].astype(np.float32).T
    avg /= H
    return out, avg


def kernel(query, key, value, q_w, q_b, k_w, k_b, v_w, v_b,
           o_w_mean, o_w_lgstd, eps):
    _ensure_env()
    from concourse.bass_utils import run_bass_kernel_spmd

    if "nc" not in _cache:
        _cache["nc"] = build_nc()
    nc = _cache["nc"]

    args = [query, key, value, q_w, q_b, k_w, k_b, v_w, v_b,
            o_w_mean, o_w_lgstd, eps]
    args = [np.asarray(a, np.float32) for a in args]
    in_maps = make_in_maps(*args)
    res = run_bass_kernel_spmd(nc, in_maps, core_ids=list(range(NCORES)))
    return assemble(res.results)
